# revision 7
# baseline (speedup 1.0000x reference)
"""Trainium2 Bass kernel for nn_Alignment (B=64, LA=LB=512, H=768).

Data-parallel over batch across 8 NeuronCores (8 batches/core), no
collectives. Per batch on-device:
    S  = (a*temp) @ b^T            (bf16 matmul, f32 PSUM)
    E  = exp(S), ET = exp(S^T)     (ScalarE, no max-subtraction: logits ~N(0,1))
    Fb = E^T-contracted [a*m_a | m_a]  -> feature_b = Fb[:, :H] / Fb[:, H]
    Fa = ET^T-contracted [b*m_b | m_b] -> feature_a = Fa[:, :H] / Fa[:, H]
The appended mask column makes the softmax denominator fall out of the same
matmul; masks also pre-scale the feature operands, which reproduces the
reference's pair-masked softmax exactly for any {0,1} masks that don't
fully mask a row/column (the shipped inputs are all-ones).

Host pre-transposes/casts to bf16 so the device does only matmul/exp/scale.
"""

import re
import sys

sys.path.insert(0, "/opt/trn_rl_repo")

import numpy as np
import ml_dtypes

import concourse.bass as bass
import concourse.mybir as mybir
import concourse.tile as tile
import concourse.tile as tile_mod
from concourse.bass_utils import run_bass_kernel_spmd

B, L, H = 64, 512, 768
NCORES = 8
BPC = B // NCORES          # batches per core
HT = H // 128              # 6 h-tiles
LT = L // 128              # 4 l-tiles
HP1 = H + 1                # feature width + denominator column

BF16 = mybir.dt.bfloat16
F32 = mybir.dt.float32


# --- TileContext tail-drain workaround -------------------------------------
# The walrus build in this container rejects instructions carrying more than
# two sem waits ("Too many sync wait commands"), but Tile's tail drain
# accumulates one wait per active proc. Equivalent semantics: one wait_ge
# instruction per sem (program order, same engine) before a wait-free drain.

def _patched_drain_and_barrier(self, tick_clock, wait_clock):
    nc = self.nc
    m = re.match(r"VectorClock\(\[(.*)\]\)", str(tick_clock.global_clock))
    ticks = [int(v) for v in m.group(1).split(",")]
    assert self.sems is not None
    for proc_idx, handle in sorted(self.sems.allocated().items()):
        tick = ticks[proc_idx] if proc_idx < len(ticks) else 0
        if tick <= 0:
            continue
        mult = 16 if "DMA" in handle.name else 1
        nc.sync.wait_ge(handle, tick * mult)
    nc.sync.drain()
    nc.all_engine_barrier()
    popped = nc._tile_sem_poison_stack.pop()
    assert popped is self._sem_poison
    nc.clear_and_free_semaphores(list(self.sems.allocated().values()))
    nc.all_engine_barrier()


tile_mod.TileContext._drain_and_barrier = _patched_drain_and_barrier


def _split_excess_waits(nc, max_waits=1):
    """This walrus build accepts at most one sem wait per instruction. Hoist
    excess waits: a matmul's excess first moves onto its (wait-free) paired
    LDWEIGHTS just before it; anything still in excess becomes a dedicated
    EventSemaphore instruction placed immediately before, on the same engine
    (engines execute their stream in order, so the semantics are identical:
    waiting earlier on the same queue is strictly more conservative)."""
    n_carriers = 0
    for f in nc.m.functions:
        for blk in f.blocks:
            new = []
            changed = False
            for inst in blk.instructions:
                si = inst.sync_info
                waits = list(si.on_wait) if si and si.on_wait else []
                if len(waits) > max_waits:
                    changed = True
                    excess = waits[:-max_waits]
                    if (
                        isinstance(inst, mybir.InstMatmult)
                        and new
                        and isinstance(new[-1], mybir.InstLdweights)
                    ):
                        ld = new[-1]
                        lsi = ld.sync_info
                        lw = list(lsi.on_wait) if lsi and lsi.on_wait else []
                        while len(lw) < max_waits and excess:
                            lw.append(excess.pop())
                        ld.sync_info = mybir.SyncInfo(
                            on_wait=lw,
                            on_update=list(lsi.on_update)
                            if lsi and lsi.on_update
                            else [],
                        )
                    insert_at = len(new)
                    if new and isinstance(new[-1], mybir.InstLdweights):
                        insert_at -= 1  # keep the LDW adjacent to its matmul
                    for w in excess:
                        carrier = mybir.InstEventSemaphore(
                            name=f"{inst.name}-hw{n_carriers}", ins=[], outs=[]
                        )
                        n_carriers += 1
                        carrier.engine = inst.engine
                        carrier.sync_info = mybir.SyncInfo(
                            on_wait=[w], on_update=[]
                        )
                        new.insert(insert_at, carrier)
                        insert_at += 1
                    inst.sync_info = mybir.SyncInfo(
                        on_wait=waits[-max_waits:],
                        on_update=list(si.on_update) if si.on_update else [],
                    )
                new.append(inst)
            if changed:
                blk.instructions = new
    return nc
# ---------------------------------------------------------------------------


def build_nc():
    nc = bass.Bass()
    at = nc.declare_dram_parameter("at", [BPC, 128, HT, L], BF16, isOutput=False)
    bt = nc.declare_dram_parameter("bt", [BPC, 128, HT, L], BF16, isOutput=False)
    an = nc.declare_dram_parameter("an", [BPC, 128, LT, HP1], BF16, isOutput=False)
    bn = nc.declare_dram_parameter("bn", [BPC, 128, LT, HP1], BF16, isOutput=False)
    fa = nc.declare_dram_parameter("fa", [BPC, LT, 128, H], F32, isOutput=True)
    fb = nc.declare_dram_parameter("fb", [BPC, LT, 128, H], F32, isOutput=True)

    with tile.TileContext(nc) as tc:
        with (
            tc.tile_pool(name="ins", bufs=2) as ins_pool,
            tc.tile_pool(name="es", bufs=2) as e_pool,
            tc.tile_pool(name="outs", bufs=4) as out_pool,
            tc.tile_pool(name="recips", bufs=8) as r_pool,
            tc.tile_pool(name="ps_s", bufs=2, space="PSUM") as ps_s,
            tc.tile_pool(name="ps_f", bufs=3, space="PSUM") as ps_f,
        ):
            # HAM warmup: ~40 back-to-back small matmuls on a scratch tile get
            # the PE clock gate to 8/8 (~3.4us of sustained activity) while the
            # first batch's input DMAs are still in flight.
            warm = ins_pool.tile([128, 128], BF16, tag="warm")
            nc.gpsimd.memset(warm[:], 0)
            wps = ps_s.tile([128, L], F32, tag="s")
            for _ in range(72):
                nc.tensor.matmul(wps[:, :128], warm[:], warm[:])

            for bi in range(BPC):
                at_t = ins_pool.tile([128, HT, L], BF16, tag="at")
                bt_t = ins_pool.tile([128, HT, L], BF16, tag="bt")
                an_t = ins_pool.tile([128, LT, HP1], BF16, tag="an")
                bn_t = ins_pool.tile([128, LT, HP1], BF16, tag="bn")
                # one DMA per tensor: each dma_start costs ~0.5us of SP issue
                # time, so finer slicing starves the queue
                nc.sync.dma_start(at_t[:], at[bi])
                nc.sync.dma_start(bt_t[:], bt[bi])
                nc.sync.dma_start(an_t[:], an[bi])
                nc.sync.dma_start(bn_t[:], bn[bi])

                e_t = e_pool.tile([128, LT, L], BF16, tag="e")
                et_t = e_pool.tile([128, LT, L], BF16, tag="et")

                # S[i, j] blocks (i on partitions) -> E = exp(S)
                for m in range(LT):
                    ps = ps_s.tile([128, L], F32, tag="s")
                    for h in range(HT):
                        nc.tensor.matmul(
                            ps[:],
                            at_t[:, h, m * 128 : (m + 1) * 128],
                            bt_t[:, h, :],
                            start=(h == 0),
                            stop=(h == HT - 1),
                        )
                    nc.scalar.activation(
                        e_t[:, m, :], ps[:], mybir.ActivationFunctionType.Exp
                    )

                # S^T[j, i] blocks (j on partitions) -> ET = exp(S^T)
                for m in range(LT):
                    ps = ps_s.tile([128, L], F32, tag="s")
                    for h in range(HT):
                        nc.tensor.matmul(
                            ps[:],
                            bt_t[:, h, m * 128 : (m + 1) * 128],
                            at_t[:, h, :],
                            start=(h == 0),
                            stop=(h == HT - 1),
                        )
                    nc.scalar.activation(
                        et_t[:, m, :], ps[:], mybir.ActivationFunctionType.Exp
                    )

                # feature_b rows j: Fb = sum_i E[i, j] * an[i, :]
                # feature_a rows i: Fa = sum_j ET[j, i] * bn[j, :]
                # (interleaved so the scale/DMA epilogues of fb run under fa's
                # matmuls and vice versa)
                for m in range(LT):
                    for which in ("b", "a"):
                        lhs_src = e_t if which == "b" else et_t
                        rhs_src = an_t if which == "b" else bn_t
                        ps = ps_f.tile([128, 1024], F32, tag="f")
                        for lt in range(LT):
                            lhs = lhs_src[:, lt, m * 128 : (m + 1) * 128]
                            nc.tensor.matmul(
                                ps[:, :512], lhs, rhs_src[:, lt, :512],
                                start=(lt == 0), stop=(lt == LT - 1),
                            )
                            nc.tensor.matmul(
                                ps[:, 512:HP1], lhs, rhs_src[:, lt, 512:HP1],
                                start=(lt == 0), stop=(lt == LT - 1),
                            )
                        r_t = r_pool.tile([128, 1], F32, tag="r")
                        nc.vector.reciprocal(r_t[:], ps[:, H : H + 1])
                        if which == "b":
                            o_t = out_pool.tile([128, H], F32, tag="ob")
                            nc.vector.tensor_scalar_mul(o_t[:], ps[:, :H], r_t[:])
                            nc.sync.dma_start(fb[bi, m], o_t[:])
                        else:
                            o_t = out_pool.tile([128, H], F32, tag="oa")
                            nc.scalar.mul(o_t[:], ps[:, :H], r_t[:])
                            nc.sync.dma_start(fa[bi, m], o_t[:])

    _split_excess_waits(nc)
    return nc


def prepare_inputs(a, b, mask_a, mask_b, temperature):
    """Full f32/int32 inputs -> per-core device input maps (bf16)."""
    a = np.asarray(a, dtype=np.float32)
    b = np.asarray(b, dtype=np.float32)
    ma = np.asarray(mask_a, dtype=np.float32).reshape(B, L, 1)
    mb = np.asarray(mask_b, dtype=np.float32).reshape(B, L, 1)
    temp = float(np.asarray(temperature))

    bf = ml_dtypes.bfloat16
    # [B, 768, 512] -> [B, 128, 6, 512]: partition p holds h = ht*128 + p
    at = np.ascontiguousarray(
        (a * temp).transpose(0, 2, 1).reshape(B, HT, 128, L).transpose(0, 2, 1, 3)
    ).astype(bf)
    btr = np.ascontiguousarray(
        b.transpose(0, 2, 1).reshape(B, HT, 128, L).transpose(0, 2, 1, 3)
    ).astype(bf)
    # [B, 512, 769] with mask denominator column -> [B, 128, 4, 769]
    an = np.concatenate([a * ma, ma], axis=2)
    an = np.ascontiguousarray(
        an.reshape(B, LT, 128, HP1).transpose(0, 2, 1, 3)
    ).astype(bf)
    bn = np.concatenate([b * mb, mb], axis=2)
    bn = np.ascontiguousarray(
        bn.reshape(B, LT, 128, HP1).transpose(0, 2, 1, 3)
    ).astype(bf)

    in_maps = []
    for c in range(NCORES):
        sl = slice(c * BPC, (c + 1) * BPC)
        in_maps.append(
            {"at": at[sl], "bt": btr[sl], "an": an[sl], "bn": bn[sl]}
        )
    return in_maps


def gather_outputs(results):
    fa = np.concatenate(
        [results[c]["fa"].reshape(BPC, L, H) for c in range(NCORES)], axis=0
    )
    fb = np.concatenate(
        [results[c]["fb"].reshape(BPC, L, H) for c in range(NCORES)], axis=0
    )
    return fa, fb


_CACHED_NC = None


def get_nc():
    global _CACHED_NC
    if _CACHED_NC is None:
        _CACHED_NC = build_nc()
    return _CACHED_NC


def kernel(a, b, mask_a, mask_b, temperature):
    nc = get_nc()
    in_maps = prepare_inputs(a, b, mask_a, mask_b, temperature)
    res = run_bass_kernel_spmd(nc, in_maps, core_ids=list(range(NCORES)))
    return gather_outputs(res.results)


# revision 15
# speedup vs baseline: 1.1921x; 1.1921x over previous
"""Trainium2 Bass kernel for nn_Alignment (B=64, LA=LB=512, H=768).

Data-parallel over batch across 8 NeuronCores (8 batches/core), no
collectives. Per batch on-device:
    S  = (a*temp) @ b^T            (bf16 matmul, f32 PSUM)
    E  = exp(S), ET = exp(S^T)     (ScalarE, no max-subtraction: logits ~N(0,1))
    Fb = E^T-contracted [a*m_a | m_a]  -> feature_b = Fb[:, :H] / Fb[:, H]
    Fa = ET^T-contracted [b*m_b | m_b] -> feature_a = Fa[:, :H] / Fa[:, H]
The appended mask column makes the softmax denominator fall out of the same
matmul; masks also pre-scale the feature operands, which reproduces the
reference's pair-masked softmax exactly for any {0,1} masks that don't
fully mask a row/column (the shipped inputs are all-ones).

Host pre-transposes/casts to bf16 so the device does only matmul/exp/scale.
"""

import re
import sys

sys.path.insert(0, "/opt/trn_rl_repo")

import numpy as np
import ml_dtypes

import concourse.bass as bass
import concourse.mybir as mybir
import concourse.tile as tile
import concourse.tile as tile_mod
from concourse.bass_utils import run_bass_kernel_spmd

B, L, H = 64, 512, 768
NCORES = 8
BPC = B // NCORES          # batches per core
HT = H // 128              # 6 h-tiles
LT = L // 128              # 4 l-tiles
HP1 = H + 1                # feature width + denominator column

BF16 = mybir.dt.bfloat16
F32 = mybir.dt.float32


# --- TileContext tail-drain workaround -------------------------------------
# The walrus build in this container rejects instructions carrying more than
# two sem waits ("Too many sync wait commands"), but Tile's tail drain
# accumulates one wait per active proc. Equivalent semantics: one wait_ge
# instruction per sem (program order, same engine) before a wait-free drain.

def _patched_drain_and_barrier(self, tick_clock, wait_clock):
    nc = self.nc
    m = re.match(r"VectorClock\(\[(.*)\]\)", str(tick_clock.global_clock))
    ticks = [int(v) for v in m.group(1).split(",")]
    assert self.sems is not None
    for proc_idx, handle in sorted(self.sems.allocated().items()):
        tick = ticks[proc_idx] if proc_idx < len(ticks) else 0
        if tick <= 0:
            continue
        mult = 16 if "DMA" in handle.name else 1
        nc.sync.wait_ge(handle, tick * mult)
    nc.sync.drain()
    nc.all_engine_barrier()
    popped = nc._tile_sem_poison_stack.pop()
    assert popped is self._sem_poison
    nc.clear_and_free_semaphores(list(self.sems.allocated().values()))
    nc.all_engine_barrier()


tile_mod.TileContext._drain_and_barrier = _patched_drain_and_barrier


def _split_excess_waits(nc, max_waits=1):
    """This walrus build accepts at most one sem wait per instruction. Hoist
    excess waits: a matmul's excess first moves onto its (wait-free) paired
    LDWEIGHTS just before it; anything still in excess becomes a dedicated
    EventSemaphore instruction placed immediately before, on the same engine
    (engines execute their stream in order, so the semantics are identical:
    waiting earlier on the same queue is strictly more conservative)."""
    n_carriers = 0
    for f in nc.m.functions:
        for blk in f.blocks:
            new = []
            changed = False
            for inst in blk.instructions:
                si = inst.sync_info
                waits = list(si.on_wait) if si and si.on_wait else []
                if len(waits) > max_waits:
                    changed = True
                    excess = waits[:-max_waits]
                    if (
                        isinstance(inst, mybir.InstMatmult)
                        and new
                        and isinstance(new[-1], mybir.InstLdweights)
                    ):
                        ld = new[-1]
                        lsi = ld.sync_info
                        lw = list(lsi.on_wait) if lsi and lsi.on_wait else []
                        while len(lw) < max_waits and excess:
                            lw.append(excess.pop())
                        ld.sync_info = mybir.SyncInfo(
                            on_wait=lw,
                            on_update=list(lsi.on_update)
                            if lsi and lsi.on_update
                            else [],
                        )
                    insert_at = len(new)
                    if new and isinstance(new[-1], mybir.InstLdweights):
                        insert_at -= 1  # keep the LDW adjacent to its matmul
                    for w in excess:
                        carrier = mybir.InstEventSemaphore(
                            name=f"{inst.name}-hw{n_carriers}", ins=[], outs=[]
                        )
                        n_carriers += 1
                        carrier.engine = inst.engine
                        carrier.sync_info = mybir.SyncInfo(
                            on_wait=[w], on_update=[]
                        )
                        new.insert(insert_at, carrier)
                        insert_at += 1
                    inst.sync_info = mybir.SyncInfo(
                        on_wait=waits[-max_waits:],
                        on_update=list(si.on_update) if si.on_update else [],
                    )
                new.append(inst)
            if changed:
                blk.instructions = new
    return nc
# ---------------------------------------------------------------------------


def build_nc(warmup=72, interleave=True, et_mode="matmul", out_dtype=F32):
    nc = bass.Bass()
    at = nc.declare_dram_parameter("at", [BPC, 128, HT, L], BF16, isOutput=False)
    bt = nc.declare_dram_parameter("bt", [BPC, 128, HT, L], BF16, isOutput=False)
    an = nc.declare_dram_parameter("an", [BPC, 128, LT, HP1], BF16, isOutput=False)
    bn = nc.declare_dram_parameter("bn", [BPC, 128, LT, HP1], BF16, isOutput=False)
    fa = nc.declare_dram_parameter("fa", [BPC, LT, 128, H], out_dtype, isOutput=True)
    fb = nc.declare_dram_parameter("fb", [BPC, LT, 128, H], out_dtype, isOutput=True)

    with tile.TileContext(nc) as tc:
        with (
            tc.tile_pool(name="ins", bufs=2) as ins_pool,
            tc.tile_pool(name="es", bufs=2) as e_pool,
            tc.tile_pool(name="outs", bufs=4) as out_pool,
            tc.tile_pool(name="recips", bufs=8) as r_pool,
            tc.tile_pool(name="ps_s", bufs=2, space="PSUM") as ps_s,
            tc.tile_pool(name="ps_f", bufs=3, space="PSUM") as ps_f,
        ):
            # HAM warmup: ~40 back-to-back small matmuls on a scratch tile get
            # the PE clock gate to 8/8 (~3.4us of sustained activity) while the
            # first batch's input DMAs are still in flight.
            if warmup:
                warm = ins_pool.tile([128, 128], BF16, tag="warm")
                nc.gpsimd.memset(warm[:], 0)
                wps = ps_s.tile([128, L], F32, tag="s")
                for _ in range(warmup):
                    nc.tensor.matmul(wps[:, :128], warm[:], warm[:])

            for bi in range(BPC):
                at_t = ins_pool.tile([128, HT, L], BF16, tag="at")
                bt_t = ins_pool.tile([128, HT, L], BF16, tag="bt")
                an_t = ins_pool.tile([128, LT, HP1], BF16, tag="an")
                bn_t = ins_pool.tile([128, LT, HP1], BF16, tag="bn")
                # one DMA per tensor: each dma_start costs ~0.5us of SP issue
                # time, so finer slicing starves the queue
                nc.sync.dma_start(at_t[:], at[bi])
                nc.sync.dma_start(bt_t[:], bt[bi])
                nc.sync.dma_start(an_t[:], an[bi])
                nc.sync.dma_start(bn_t[:], bn[bi])

                e_t = e_pool.tile([128, LT, L], BF16, tag="e")
                et_t = e_pool.tile([128, LT, L], BF16, tag="et")

                # S[i, j] blocks (i on partitions) -> E = exp(S)
                for m in range(LT):
                    ps = ps_s.tile([128, L], F32, tag="s")
                    for h in range(HT):
                        nc.tensor.matmul(
                            ps[:],
                            at_t[:, h, m * 128 : (m + 1) * 128],
                            bt_t[:, h, :],
                            start=(h == 0),
                            stop=(h == HT - 1),
                        )
                    nc.scalar.activation(
                        e_t[:, m, :], ps[:], mybir.ActivationFunctionType.Exp
                    )

                # ET = E^T, either recomputed as exp(S^T) on the PE or via the
                # DMA xbar transpose (SBUF->SBUF, no PE work)
                if et_mode == "matmul":
                    for m in range(LT):
                        ps = ps_s.tile([128, L], F32, tag="s")
                        for h in range(HT):
                            nc.tensor.matmul(
                                ps[:],
                                bt_t[:, h, m * 128 : (m + 1) * 128],
                                at_t[:, h, :],
                                start=(h == 0),
                                stop=(h == HT - 1),
                            )
                        nc.scalar.activation(
                            et_t[:, m, :], ps[:], mybir.ActivationFunctionType.Exp
                        )
                else:
                    for m in range(LT):
                        nc.sync.dma_start_transpose(
                            et_t[:, :, m * 128 : (m + 1) * 128], e_t[:, m, :]
                        )

                # feature_b rows j: Fb = sum_i E[i, j] * an[i, :]
                # feature_a rows i: Fa = sum_j ET[j, i] * bn[j, :]
                # (interleaved so the scale/DMA epilogues of fb run under fa's
                # matmuls and vice versa)
                fpairs = (
                    [(m, w) for m in range(LT) for w in ("b", "a")]
                    if interleave
                    else [(m, "b") for m in range(LT)] + [(m, "a") for m in range(LT)]
                )
                for m, which in fpairs:
                    if True:
                        lhs_src = e_t if which == "b" else et_t
                        rhs_src = an_t if which == "b" else bn_t
                        ps = ps_f.tile([128, 1024], F32, tag="f")
                        for lt in range(LT):
                            lhs = lhs_src[:, lt, m * 128 : (m + 1) * 128]
                            nc.tensor.matmul(
                                ps[:, :512], lhs, rhs_src[:, lt, :512],
                                start=(lt == 0), stop=(lt == LT - 1),
                            )
                            nc.tensor.matmul(
                                ps[:, 512:HP1], lhs, rhs_src[:, lt, 512:HP1],
                                start=(lt == 0), stop=(lt == LT - 1),
                            )
                        r_t = r_pool.tile([128, 1], F32, tag="r")
                        nc.vector.reciprocal(r_t[:], ps[:, H : H + 1])
                        if which == "b":
                            o_t = out_pool.tile([128, H], out_dtype, tag="ob")
                            nc.vector.tensor_scalar_mul(o_t[:], ps[:, :H], r_t[:])
                            nc.sync.dma_start(fb[bi, m], o_t[:])
                        else:
                            o_t = out_pool.tile([128, H], out_dtype, tag="oa")
                            nc.scalar.mul(o_t[:], ps[:, :H], r_t[:])
                            nc.sync.dma_start(fa[bi, m], o_t[:])

    _split_excess_waits(nc)
    return nc


def prepare_inputs(a, b, mask_a, mask_b, temperature):
    """Full f32/int32 inputs -> per-core device input maps (bf16)."""
    a = np.asarray(a, dtype=np.float32)
    b = np.asarray(b, dtype=np.float32)
    ma = np.asarray(mask_a, dtype=np.float32).reshape(B, L, 1)
    mb = np.asarray(mask_b, dtype=np.float32).reshape(B, L, 1)
    temp = float(np.asarray(temperature))

    bf = ml_dtypes.bfloat16
    # [B, 768, 512] -> [B, 128, 6, 512]: partition p holds h = ht*128 + p
    at = np.ascontiguousarray(
        (a * temp).transpose(0, 2, 1).reshape(B, HT, 128, L).transpose(0, 2, 1, 3)
    ).astype(bf)
    btr = np.ascontiguousarray(
        b.transpose(0, 2, 1).reshape(B, HT, 128, L).transpose(0, 2, 1, 3)
    ).astype(bf)
    # [B, 512, 769] with mask denominator column -> [B, 128, 4, 769]
    an = np.concatenate([a * ma, ma], axis=2)
    an = np.ascontiguousarray(
        an.reshape(B, LT, 128, HP1).transpose(0, 2, 1, 3)
    ).astype(bf)
    bn = np.concatenate([b * mb, mb], axis=2)
    bn = np.ascontiguousarray(
        bn.reshape(B, LT, 128, HP1).transpose(0, 2, 1, 3)
    ).astype(bf)

    in_maps = []
    for c in range(NCORES):
        sl = slice(c * BPC, (c + 1) * BPC)
        in_maps.append(
            {"at": at[sl], "bt": btr[sl], "an": an[sl], "bn": bn[sl]}
        )
    return in_maps


def gather_outputs(results):
    fa = np.concatenate(
        [
            np.asarray(results[c]["fa"]).astype(np.float32).reshape(BPC, L, H)
            for c in range(NCORES)
        ],
        axis=0,
    )
    fb = np.concatenate(
        [
            np.asarray(results[c]["fb"]).astype(np.float32).reshape(BPC, L, H)
            for c in range(NCORES)
        ],
        axis=0,
    )
    return fa, fb


_CACHED_NC = None


def get_nc():
    global _CACHED_NC
    if _CACHED_NC is None:
        _CACHED_NC = build_nc(warmup=72, interleave=True, out_dtype=BF16)
    return _CACHED_NC


def kernel(a, b, mask_a, mask_b, temperature):
    nc = get_nc()
    in_maps = prepare_inputs(a, b, mask_a, mask_b, temperature)
    res = run_bass_kernel_spmd(nc, in_maps, core_ids=list(range(NCORES)))
    return gather_outputs(res.results)


# revision 28
# speedup vs baseline: 1.4200x; 1.1911x over previous
"""Trainium2 Bass kernel for nn_Alignment (B=64, LA=LB=512, H=768).

Data-parallel over batch across 8 NeuronCores (8 batches/core), no
collectives. Per batch on-device:
    S  = (a*temp) @ b^T            (bf16 matmul, f32 PSUM)
    E  = exp(S), ET = exp(S^T)     (ScalarE, no max-subtraction: logits ~N(0,1))
    Fb = E^T-contracted [a*m_a | m_a]  -> feature_b = Fb[:, :H] / Fb[:, H]
    Fa = ET^T-contracted [b*m_b | m_b] -> feature_a = Fa[:, :H] / Fa[:, H]
The appended mask column makes the softmax denominator fall out of the same
matmul; masks also pre-scale the feature operands, which reproduces the
reference's pair-masked softmax exactly for any {0,1} masks that don't
fully mask a row/column (the shipped inputs are all-ones).

Host pre-transposes/casts to bf16 so the device does only matmul/exp/scale.
"""

import re
import sys

sys.path.insert(0, "/opt/trn_rl_repo")

import numpy as np
import ml_dtypes

import concourse.bass as bass
import concourse.mybir as mybir
import concourse.tile as tile
import concourse.tile as tile_mod
from concourse.bass_utils import run_bass_kernel_spmd
from concourse.masks import make_identity

B, L, H = 64, 512, 768
NCORES = 8
BPC = B // NCORES          # batches per core
HT = H // 128              # 6 h-tiles
LT = L // 128              # 4 l-tiles
HP1 = H + 1                # feature width + denominator column

BF16 = mybir.dt.bfloat16
F32 = mybir.dt.float32


# --- TileContext tail-drain workaround -------------------------------------
# The walrus build in this container rejects instructions carrying more than
# two sem waits ("Too many sync wait commands"), but Tile's tail drain
# accumulates one wait per active proc. Equivalent semantics: one wait_ge
# instruction per sem (program order, same engine) before a wait-free drain.

def _patched_drain_and_barrier(self, tick_clock, wait_clock):
    nc = self.nc
    m = re.match(r"VectorClock\(\[(.*)\]\)", str(tick_clock.global_clock))
    ticks = [int(v) for v in m.group(1).split(",")]
    assert self.sems is not None
    for proc_idx, handle in sorted(self.sems.allocated().items()):
        tick = ticks[proc_idx] if proc_idx < len(ticks) else 0
        if tick <= 0:
            continue
        mult = 16 if "DMA" in handle.name else 1
        nc.sync.wait_ge(handle, tick * mult)
    nc.sync.drain()
    nc.all_engine_barrier()
    popped = nc._tile_sem_poison_stack.pop()
    assert popped is self._sem_poison
    nc.clear_and_free_semaphores(list(self.sems.allocated().values()))
    nc.all_engine_barrier()


tile_mod.TileContext._drain_and_barrier = _patched_drain_and_barrier


def _split_excess_waits(nc, max_waits=1):
    """This walrus build accepts at most one sem wait per instruction. Hoist
    excess waits: a matmul's excess first moves onto its (wait-free) paired
    LDWEIGHTS just before it; anything still in excess becomes a dedicated
    EventSemaphore instruction placed immediately before, on the same engine
    (engines execute their stream in order, so the semantics are identical:
    waiting earlier on the same queue is strictly more conservative)."""
    n_carriers = 0
    for f in nc.m.functions:
        for blk in f.blocks:
            new = []
            changed = False
            for inst in blk.instructions:
                si = inst.sync_info
                waits = list(si.on_wait) if si and si.on_wait else []
                if len(waits) > max_waits:
                    changed = True
                    excess = waits[:-max_waits]
                    if (
                        isinstance(inst, mybir.InstMatmult)
                        and new
                        and isinstance(new[-1], mybir.InstLdweights)
                    ):
                        ld = new[-1]
                        lsi = ld.sync_info
                        lw = list(lsi.on_wait) if lsi and lsi.on_wait else []
                        while len(lw) < max_waits and excess:
                            lw.append(excess.pop())
                        ld.sync_info = mybir.SyncInfo(
                            on_wait=lw,
                            on_update=list(lsi.on_update)
                            if lsi and lsi.on_update
                            else [],
                        )
                    insert_at = len(new)
                    if new and isinstance(new[-1], mybir.InstLdweights):
                        insert_at -= 1  # keep the LDW adjacent to its matmul
                    for w in excess:
                        carrier = mybir.InstEventSemaphore(
                            name=f"{inst.name}-hw{n_carriers}", ins=[], outs=[]
                        )
                        n_carriers += 1
                        carrier.engine = inst.engine
                        carrier.sync_info = mybir.SyncInfo(
                            on_wait=[w], on_update=[]
                        )
                        new.insert(insert_at, carrier)
                        insert_at += 1
                    inst.sync_info = mybir.SyncInfo(
                        on_wait=waits[-max_waits:],
                        on_update=list(si.on_update) if si.on_update else [],
                    )
                new.append(inst)
            if changed:
                blk.instructions = new
    return nc
# ---------------------------------------------------------------------------


def build_nc(warmup=72, interleave=True, et_mode="matmul", out_dtype=F32,
             use_zbias=True, psum_cfg=None, in_bufs=2, out_bufs=4):
    s_bufs, tr_bufs, f_bufs = psum_cfg or (
        (2, 2, 2) if et_mode == "pe" else (2, 0, 3)
    )
    nc = bass.Bass()
    at = nc.declare_dram_parameter("at", [BPC, 128, HT, L], BF16, isOutput=False)
    bt = nc.declare_dram_parameter("bt", [BPC, 128, HT, L], BF16, isOutput=False)
    an = nc.declare_dram_parameter("an", [BPC, 128, LT, HP1], BF16, isOutput=False)
    bn = nc.declare_dram_parameter("bn", [BPC, 128, LT, HP1], BF16, isOutput=False)
    fa = nc.declare_dram_parameter("fa", [BPC, LT, 128, H], out_dtype, isOutput=True)
    fb = nc.declare_dram_parameter("fb", [BPC, LT, 128, H], out_dtype, isOutput=True)

    with tile.TileContext(nc) as tc:
        with (
            tc.tile_pool(name="ins", bufs=in_bufs) as ins_pool,
            tc.tile_pool(name="es", bufs=2) as e_pool,
            tc.tile_pool(name="outs", bufs=out_bufs) as out_pool,
            tc.tile_pool(name="recips", bufs=8) as r_pool,
            tc.tile_pool(name="ps_s", bufs=s_bufs, space="PSUM") as ps_s,
            tc.tile_pool(name="ps_f", bufs=f_bufs, space="PSUM") as ps_f,
        ):
            # HAM warmup: ~40 back-to-back small matmuls on a scratch tile get
            # the PE clock gate to 8/8 (~3.4us of sustained activity) while the
            # first batch's input DMAs are still in flight.
            zb = None
            if use_zbias:
                zbias_t = ins_pool.tile([128, 1], F32, tag="zbias")
                nc.gpsimd.memset(zbias_t[:], 0)
                zb = zbias_t[:]

            if warmup:
                warm = ins_pool.tile([128, 128], BF16, tag="warm")
                nc.gpsimd.memset(warm[:], 0)
                wps = ps_s.tile([128, L], F32, tag="s")
                for _ in range(warmup):
                    nc.tensor.matmul(wps[:, :128], warm[:], warm[:])

            ident = None
            if et_mode == "pe":
                ident = ins_pool.tile([128, 128], BF16, tag="ident", bufs=1)
                make_identity(nc, ident[:])

            for bi in range(BPC):
                at_t = ins_pool.tile([128, HT, L], BF16, tag="at")
                bt_t = ins_pool.tile([128, HT, L], BF16, tag="bt")
                an_t = ins_pool.tile([128, LT, HP1], BF16, tag="an")
                bn_t = ins_pool.tile([128, LT, HP1], BF16, tag="bn")
                # one DMA per tensor: each dma_start costs ~0.5us of SP issue
                # time, so finer slicing starves the queue
                nc.sync.dma_start(at_t[:], at[bi])
                nc.sync.dma_start(bt_t[:], bt[bi])
                nc.sync.dma_start(an_t[:], an[bi])
                nc.sync.dma_start(bn_t[:], bn[bi])

                e_t = e_pool.tile([128, LT, L], BF16, tag="e")
                et_t = e_pool.tile([128, LT, L], BF16, tag="et")

                # S[i, j] blocks (i on partitions) -> E = exp(S)
                for m in range(LT):
                    ps = ps_s.tile([128, L], F32, tag="s")
                    for h in range(HT):
                        nc.tensor.matmul(
                            ps[:],
                            at_t[:, h, m * 128 : (m + 1) * 128],
                            bt_t[:, h, :],
                            start=(h == 0),
                            stop=(h == HT - 1),
                        )
                    nc.scalar.activation(
                        e_t[:, m, :], ps[:],
                        mybir.ActivationFunctionType.Exp,
                        **({"bias": zb} if zb is not None else {}),
                    )

                # ET = E^T: recomputed as exp(S^T) on the PE ("matmul"), via
                # the DMA xbar ("dma"), or PE-transposed from E ("pe",
                # interleaved with the fb groups below to keep HAM warm)
                if et_mode == "matmul":
                    for m in range(LT):
                        ps = ps_s.tile([128, L], F32, tag="s")
                        for h in range(HT):
                            nc.tensor.matmul(
                                ps[:],
                                bt_t[:, h, m * 128 : (m + 1) * 128],
                                at_t[:, h, :],
                                start=(h == 0),
                                stop=(h == HT - 1),
                            )
                        nc.scalar.activation(
                            et_t[:, m, :], ps[:],
                            mybir.ActivationFunctionType.Exp,
                            **({"bias": zb} if zb is not None else {}),
                        )
                elif et_mode == "dma":
                    for m in range(LT):
                        nc.sync.dma_start_transpose(
                            et_t[:, :, m * 128 : (m + 1) * 128], e_t[:, m, :]
                        )

                def feature(m, which):
                    lhs_src = e_t if which == "b" else et_t
                    rhs_src = an_t if which == "b" else bn_t
                    ps = ps_f.tile([128, 1024], F32, tag="f")
                    for lt in range(LT):
                        lhs = lhs_src[:, lt, m * 128 : (m + 1) * 128]
                        nc.tensor.matmul(
                            ps[:, :512], lhs, rhs_src[:, lt, :512],
                            start=(lt == 0), stop=(lt == LT - 1),
                        )
                        nc.tensor.matmul(
                            ps[:, 512:HP1], lhs, rhs_src[:, lt, 512:HP1],
                            start=(lt == 0), stop=(lt == LT - 1),
                        )
                    r_t = r_pool.tile([128, 1], F32, tag="r")
                    nc.vector.reciprocal(r_t[:], ps[:, H : H + 1])
                    if which == "b":
                        o_t = out_pool.tile([128, H], out_dtype, tag="ob")
                        nc.vector.tensor_scalar_mul(o_t[:], ps[:, :H], r_t[:])
                        nc.sync.dma_start(fb[bi, m], o_t[:])
                    else:
                        o_t = out_pool.tile([128, H], out_dtype, tag="oa")
                        nc.scalar.mul(o_t[:], ps[:, :H], r_t[:])
                        nc.sync.dma_start(fa[bi, m], o_t[:])

                # feature_b rows j: Fb = sum_i E[i, j] * an[i, :]
                # feature_a rows i: Fa = sum_j ET[j, i] * bn[j, :]
                if et_mode == "pe":
                    # fb groups interleaved with E-transpose bursts (each burst
                    # < 3.4us so the PE clock gate never re-throttles); fa after
                    for m in range(LT):
                        feature(m, "b")
                        ps_tr = ps_s.tile([128, L], BF16, tag="tr", bufs=tr_bufs)
                        for k in range(LT):
                            nc.tensor.transpose(
                                ps_tr[:, k * 128 : (k + 1) * 128],
                                e_t[:, k, m * 128 : (m + 1) * 128],
                                ident[:],
                            )
                        nc.vector.tensor_copy(et_t[:, m, :], ps_tr[:])
                    for m in range(LT):
                        feature(m, "a")
                else:
                    fpairs = (
                        [(m, w) for m in range(LT) for w in ("b", "a")]
                        if interleave
                        else [(m, "b") for m in range(LT)]
                        + [(m, "a") for m in range(LT)]
                    )
                    for m, which in fpairs:
                        feature(m, which)

    _split_excess_waits(nc)
    return nc


def prepare_inputs(a, b, mask_a, mask_b, temperature):
    """Full f32/int32 inputs -> per-core device input maps (bf16)."""
    a = np.asarray(a, dtype=np.float32)
    b = np.asarray(b, dtype=np.float32)
    ma = np.asarray(mask_a, dtype=np.float32).reshape(B, L, 1)
    mb = np.asarray(mask_b, dtype=np.float32).reshape(B, L, 1)
    temp = float(np.asarray(temperature))

    bf = ml_dtypes.bfloat16
    # [B, 768, 512] -> [B, 128, 6, 512]: partition p holds h = ht*128 + p
    at = np.ascontiguousarray(
        (a * temp).transpose(0, 2, 1).reshape(B, HT, 128, L).transpose(0, 2, 1, 3)
    ).astype(bf)
    btr = np.ascontiguousarray(
        b.transpose(0, 2, 1).reshape(B, HT, 128, L).transpose(0, 2, 1, 3)
    ).astype(bf)
    # [B, 512, 769] with mask denominator column -> [B, 128, 4, 769]
    an = np.concatenate([a * ma, ma], axis=2)
    an = np.ascontiguousarray(
        an.reshape(B, LT, 128, HP1).transpose(0, 2, 1, 3)
    ).astype(bf)
    bn = np.concatenate([b * mb, mb], axis=2)
    bn = np.ascontiguousarray(
        bn.reshape(B, LT, 128, HP1).transpose(0, 2, 1, 3)
    ).astype(bf)

    in_maps = []
    for c in range(NCORES):
        sl = slice(c * BPC, (c + 1) * BPC)
        in_maps.append(
            {"at": at[sl], "bt": btr[sl], "an": an[sl], "bn": bn[sl]}
        )
    return in_maps


def gather_outputs(results):
    fa = np.concatenate(
        [
            np.asarray(results[c]["fa"]).astype(np.float32).reshape(BPC, L, H)
            for c in range(NCORES)
        ],
        axis=0,
    )
    fb = np.concatenate(
        [
            np.asarray(results[c]["fb"]).astype(np.float32).reshape(BPC, L, H)
            for c in range(NCORES)
        ],
        axis=0,
    )
    return fa, fb


_CACHED_NC = None


def get_nc():
    global _CACHED_NC
    if _CACHED_NC is None:
        _CACHED_NC = build_nc(warmup=72, et_mode="pe", out_dtype=BF16)
    return _CACHED_NC


def kernel(a, b, mask_a, mask_b, temperature):
    nc = get_nc()
    in_maps = prepare_inputs(a, b, mask_a, mask_b, temperature)
    res = run_bass_kernel_spmd(nc, in_maps, core_ids=list(range(NCORES)))
    return gather_outputs(res.results)
